# revision 14
# baseline (speedup 1.0000x reference)
"""NanoGPT (GPT-2 124M) forward pass on 8 Trainium2 NeuronCores.

Sharding: token-parallel. B*T = 2048 tokens split into 8 contiguous chunks of
256; cores 0-3 hold batch 0, cores 4-7 hold batch 1. Weights are replicated
(bf16). Per layer each core computes qkv / attention / MLP for its 256 tokens;
K (feature-major) and V (token-major) are all-gathered within each 4-core
batch group so every core can attend over its full causal prefix (causal
structure applied via a per-core multiplicative mask input). lm_head is
vocab-parallel: after a final 8-rank all-gather of the lnf output, each core
computes logits for all 2048 tokens x a 6656-wide vocab shard of the (tied,
transposed) embedding matrix.

On-device layout: activations are feature-major [128 partitions, C/128, 256
tokens] so no transposes are ever needed; LayerNorm statistics are computed
with ones-vector matmuls (partition-axis sums), rsqrt via exp(-0.5*ln(v+eps)),
and per-token scalars are broadcast across partitions with gpsimd
partition_broadcast. Matmul inputs are bf16 (full PE rate); PSUM accumulation
and the residual stream are fp32.
"""

import os
import numpy as np
import ml_dtypes

import concourse.bass as bass
import concourse.mybir as mybir
import concourse.tile as tile
from concourse import bacc, bass_utils

# ---- model dims (hardcoded per problem spec) ----
B, T, C, H, L, V = 2, 1024, 768, 12, 12, 50257
D = C // H            # 64
NC = 8                # cores
TL = (B * T) // NC    # 256 local tokens per core
S = C // 128          # 6 feature sub-blocks
SQK = 2 * S           # 12 q+k feature blocks
KT = 1024 // 128      # 8 key-token blocks per batch
VSH = 6656            # padded vocab shard per core (13*512); 8*6656 >= V
EPS = 1e-5
ATT_SCALE = 1.0 / np.sqrt(D)

f32 = mybir.dt.float32
f32r = mybir.dt.float32r
bf16 = mybir.dt.bfloat16
AF = mybir.ActivationFunctionType
OP = mybir.AluOpType

RG4 = [[0, 1, 2, 3], [4, 5, 6, 7]]
RG8 = [[0, 1, 2, 3, 4, 5, 6, 7]]

_CACHE = {}


def _r(dt_np):
    return mybir.dt.from_np(np.dtype(dt_np))


def _emit(nc, tc, tens, n_layers):
    """Emit the full per-core program."""
    from contextlib import ExitStack

    with ExitStack() as octx, ExitStack() as ctx:
        dram_keep = octx.enter_context(tc.tile_pool(name="dram_keep", bufs=1, space="DRAM"))
        # ---------------- pools ----------------
        singles = ctx.enter_context(tc.tile_pool(name="singles", bufs=1))
        wq = ctx.enter_context(tc.tile_pool(name="wq", bufs=3))       # qk weight chunks
        wv = ctx.enter_context(tc.tile_pool(name="wv", bufs=2))       # v weight
        wp = ctx.enter_context(tc.tile_pool(name="wp", bufs=3))       # proj chunks
        wf = ctx.enter_context(tc.tile_pool(name="wf", bufs=3))       # fc chunks
        wo = ctx.enter_context(tc.tile_pool(name="wo", bufs=3))       # fco chunks
        act = ctx.enter_context(tc.tile_pool(name="act", bufs=1))     # f32 LN temps
        actb = ctx.enter_context(tc.tile_pool(name="actb", bufs=1))   # bf16 acts
        kv = ctx.enter_context(tc.tile_pool(name="kv", bufs=1))       # gathered K/V
        attp = ctx.enter_context(tc.tile_pool(name="attp", bufs=2))   # attT tiles
        small = ctx.enter_context(tc.tile_pool(name="small", bufs=9)) # [1,TL] stats
        bc = ctx.enter_context(tc.tile_pool(name="bc", bufs=2))       # bcast tiles
        ps_mm = ctx.enter_context(tc.tile_pool(name="ps_mm", bufs=3, space="PSUM"))
        ps_sc = ctx.enter_context(tc.tile_pool(name="ps_sc", bufs=2, space="PSUM"))
        ps_av = ctx.enter_context(tc.tile_pool(name="ps_av", bufs=1, space="PSUM"))
        dram = ctx.enter_context(tc.tile_pool(name="dram", bufs=2, space="DRAM"))

        # ---------------- resident constants ----------------
        def load_param(name, shape_dram_ap, sbuf_shape, dtype=f32):
            t = singles.tile(sbuf_shape, dtype, tag=name)
            nc.sync.dma_start(t, shape_dram_ap)
            return t

        p128 = dict(p=128)
        ln1w = load_param("ln1w", tens["ln1_w"].rearrange("l (s p) -> p l s", **p128), [128, n_layers, S])
        ln1b = load_param("ln1b", tens["ln1_b"].rearrange("l (s p) -> p l s", **p128), [128, n_layers, S])
        ln2w = load_param("ln2w", tens["ln2_w"].rearrange("l (s p) -> p l s", **p128), [128, n_layers, S])
        ln2b = load_param("ln2b", tens["ln2_b"].rearrange("l (s p) -> p l s", **p128), [128, n_layers, S])
        lnfw = load_param("lnfw", tens["lnf_w"].rearrange("(s p) -> p s", **p128), [128, S])
        lnfb = load_param("lnfb", tens["lnf_b"].rearrange("(s p) -> p s", **p128), [128, S])
        qkvb = load_param("qkvb", tens["qkv_b"][:, : 2 * C].rearrange("l (n p) -> p l n", **p128), [128, n_layers, SQK])
        vb_row = load_param("vb_row", tens["qkv_b"][None, :, 2 * C:], [1, n_layers, C])
        projb = load_param("projb", tens["proj_b"].rearrange("l (n p) -> p l n", **p128), [128, n_layers, S])
        fcb = load_param("fcb", tens["fc_b"].rearrange("l (n p) -> p l n", **p128), [128, n_layers, 4 * S])
        fcob = load_param("fcob", tens["fco_b"].rearrange("l (n p) -> p l n", **p128), [128, n_layers, S])

        mask_sb = singles.tile([128, KT, TL], bf16)
        nc.sync.dma_start(mask_sb, tens["mask"])

        ones_col = singles.tile([128, 1], bf16)
        nc.vector.memset(ones_col, 1.0)
        eps_t = singles.tile([1, 1], f32)
        nc.vector.memset(eps_t, EPS)

        # residual stream, feature-major fp32
        xT = singles.tile([128, S, TL], f32)
        # Vaug: [ktok 128, kt 8, head 12, 65] with ones column at [..,64]
        vaug = singles.tile([128, KT, H, D + 1], bf16)
        nc.vector.memset(vaug, 1.0)

        # ---------------- embedding ----------------
        tok_sb = act.tile([128, S, TL], f32, tag="tmp")
        nc.sync.dma_start(tok_sb, tens["tokT"].rearrange("(s p) t -> p s t", **p128))
        pos_sb = act.tile([128, S, TL], f32, tag="sq")
        nc.sync.dma_start(pos_sb, tens["wpeT"].rearrange("(s p) t -> p s t", **p128))
        nc.vector.tensor_add(xT, tok_sb, pos_sb)

        # ---------------- helpers ----------------
        def layer_norm(w_col, b_col, out_bf):
            """Feature-major LN of xT -> out_bf (bf16 [128,S,TL]).
            w_col/b_col: [128, S] per-partition scale/bias columns."""
            x_bf = act.tile([128, S, TL], bf16, tag="xbf")
            nc.vector.tensor_copy(x_bf, xT)
            sq = act.tile([128, S, TL], bf16, tag="sq")
            nc.vector.tensor_mul(sq, x_bf, x_bf)
            ps_s = ps_mm.tile([1, TL], f32, tag="mm")
            ps_q = ps_mm.tile([1, TL], f32, tag="mm")
            for kt in range(S):
                nc.tensor.matmul(ps_s, ones_col, x_bf[:, kt, :],
                                 start=(kt == 0), stop=(kt == S - 1))
            for kt in range(S):
                nc.tensor.matmul(ps_q, ones_col, sq[:, kt, :],
                                 start=(kt == 0), stop=(kt == S - 1))
            mean = small.tile([1, TL], f32, tag="st")
            nc.vector.tensor_scalar_mul(mean, ps_s, 1.0 / C)
            ex2 = small.tile([1, TL], f32, tag="st")
            nc.vector.tensor_scalar_mul(ex2, ps_q, 1.0 / C)
            var = small.tile([1, TL], f32, tag="st")
            nc.vector.tensor_mul(var, mean, mean)
            nc.vector.tensor_tensor(var, ex2, var, OP.subtract)
            lnv = small.tile([1, TL], f32, tag="st")
            nc.scalar.activation(lnv, var, AF.Ln, bias=eps_t)
            inv = small.tile([1, TL], f32, tag="st")
            nc.scalar.activation(inv, lnv, AF.Exp, scale=-0.5)
            mi = small.tile([1, TL], f32, tag="st")
            nc.vector.tensor_mul(mi, mean, inv)
            inv_b = bc.tile([128, TL], f32, tag="bc")
            nc.gpsimd.partition_broadcast(inv_b, inv)
            mi_b = bc.tile([128, TL], f32, tag="bc")
            nc.gpsimd.partition_broadcast(mi_b, mi)
            tmp = act.tile([128, S, TL], f32, tag="tmp")
            nc.vector.tensor_mul(tmp, xT, inv_b[:, None, :].to_broadcast((128, S, TL)))
            nc.vector.tensor_tensor(tmp, tmp, mi_b[:, None, :].to_broadcast((128, S, TL)), OP.subtract)
            for kt in range(S):
                nc.scalar.activation(out_bf[:, kt, :], tmp[:, kt, :], AF.Identity,
                                     bias=b_col[:, kt:kt + 1], scale=w_col[:, kt:kt + 1])

        # ---------------- transformer layers ----------------
        for l in range(n_layers):
            w_qkv = tens[f"qkv_w_{l}"].rearrange("(s p) n -> p s n", **p128)
            w_proj = tens[f"proj_w_{l}"].rearrange("(s p) n -> p s n", **p128)
            w_fc = tens[f"fc_w_{l}"].rearrange("(s p) n -> p s n", **p128)
            w_fco = tens[f"fco_w_{l}"].rearrange("(s p) n -> p s n", **p128)

            # -- LN1 --
            h_bf = actb.tile([128, S, TL], bf16, tag="h")
            layer_norm(ln1w[:, l, :], ln1b[:, l, :], h_bf)

            # -- qkv matmuls: K first (feeds the AllGather), then Q, then V --
            qkT = actb.tile([128, SQK, TL], bf16, tag="qk")
            for nb in list(range(S, SQK)) + list(range(S)):  # k blocks, then q
                wt = wq.tile([128, S, 256], bf16, tag="wq")
                c0 = (nb // 2) * 256
                nc.sync.dma_start(wt, w_qkv[:, :, c0:c0 + 256])
                ps = ps_mm.tile([128, TL], f32, tag="mm")
                off = (nb % 2) * 128
                for kt in range(S):
                    nc.tensor.matmul(ps, wt[:, kt, off:off + 128], h_bf[:, kt, :],
                                     start=(kt == 0), stop=(kt == S - 1))
                nc.scalar.activation(qkT[:, nb, :], ps, AF.Identity,
                                     bias=qkvb[:, l, nb:nb + 1])
                if nb == SQK - 1:
                    # kick off K all-gather as soon as k half is complete
                    k_in = dram.tile([C, TL], bf16, tag="k_in")
                    nc.sync.dma_start(k_in.rearrange("(s p) t -> p s t", **p128),
                                      qkT[:, S:SQK, :])
                    k_out = dram.tile([4 * C, TL], bf16, tag="k_out")
                    nc.gpsimd.collective_compute(
                        "AllGather", OP.bypass, replica_groups=RG4,
                        ins=[k_in[:]], outs=[k_out[:]])

            # V (token-major, form-1)
            v_sb = actb.tile([128, 2, C], bf16, tag="v")
            for vb in range(2):
                wtv = wv.tile([128, S, 384], bf16, tag="wv")
                nc.sync.dma_start(wtv, w_qkv[:, :, 2 * C + vb * 384:2 * C + (vb + 1) * 384])
                for tb in range(2):
                    ps = ps_mm.tile([128, 384], f32, tag="mm")
                    for kt in range(S):
                        nc.tensor.matmul(ps, h_bf[:, kt, tb * 128:(tb + 1) * 128],
                                         wtv[:, kt, :],
                                         start=(kt == 0), stop=(kt == S - 1))
                    nc.scalar.activation(v_sb[:, tb, vb * 384:(vb + 1) * 384], ps, AF.Copy)
            vb_b = bc.tile([128, C], f32, tag="vbias")
            nc.gpsimd.partition_broadcast(vb_b, vb_row[0:1, l, :])
            nc.vector.tensor_add(v_sb, v_sb, vb_b[:, None, :].to_broadcast((128, 2, C)))
            v_in = dram.tile([2 * 128, C], bf16, tag="v_in")
            nc.sync.dma_start(v_in.rearrange("(tb p) f -> p tb f", **p128), v_sb)
            v_out = dram.tile([8 * 128, C], bf16, tag="v_out")
            nc.gpsimd.collective_compute(
                "AllGather", OP.bypass, replica_groups=RG4,
                ins=[v_in[:]], outs=[v_out[:]])

            # -- load gathered K/V --
            k_full = kv.tile([128, 4 * S, TL], bf16, tag="kfull")
            nc.sync.dma_start(k_full, k_out.rearrange("(r s p) q -> p (r s) q", p=128, s=S))
            v_full = kv.tile([128, KT, C], bf16, tag="vfull")
            nc.sync.dma_start(v_full, v_out.rearrange("(kt p) f -> p kt f", **p128))
            for h in range(H):
                nc.vector.tensor_copy(vaug[:, :, h, 0:D], v_full[:, :, D * h:D * (h + 1)])

            # -- attention --
            y_bf = actb.tile([128, S, TL], bf16, tag="y")
            for h in range(H):
                po = 64 * (h % 2)
                sh = h // 2
                attT = attp.tile([128, KT, TL], bf16, tag="attT")
                for half in range(2):
                    ps = ps_sc.tile([128, 4, TL], f32, tag="sc")
                    for j in range(4):
                        kt = half * 4 + j
                        lhsT = k_full[po:po + 64, S * (kt // 2) + sh, (kt % 2) * 128:(kt % 2) * 128 + 128]
                        rhs = qkT[po:po + 64, sh, :]
                        nc.tensor.matmul(ps[:, j, :], lhsT, rhs, start=True, stop=True)
                    nc.scalar.activation(attT[:, half * 4:half * 4 + 4, :], ps, AF.Exp,
                                         scale=ATT_SCALE)
                nc.vector.tensor_mul(attT, attT, mask_sb)
                ps_y = ps_av.tile([D + 1, TL], f32, tag="av")
                for kt in range(KT):
                    nc.tensor.matmul(ps_y, vaug[:, kt, h, :], attT[:, kt, :],
                                     start=(kt == 0), stop=(kt == KT - 1))
                inv_h = small.tile([1, TL], f32, tag="st")
                nc.vector.reciprocal(inv_h, ps_y[D:D + 1, :])
                inv_hb = bc.tile([64, TL], f32, tag="bch")
                nc.gpsimd.partition_broadcast(inv_hb, inv_h)
                nc.vector.tensor_mul(y_bf[po:po + 64, sh, :], ps_y[0:D, :], inv_hb)

            # -- proj + residual --
            for c in range(3):
                wt = wp.tile([128, S, 256], bf16, tag="wp")
                nc.sync.dma_start(wt, w_proj[:, :, c * 256:(c + 1) * 256])
                for j in range(2):
                    nb = 2 * c + j
                    ps = ps_mm.tile([128, TL], f32, tag="mm")
                    for kt in range(S):
                        nc.tensor.matmul(ps, wt[:, kt, j * 128:j * 128 + 128], y_bf[:, kt, :],
                                         start=(kt == 0), stop=(kt == S - 1))
                    nc.vector.tensor_add(xT[:, nb, :], xT[:, nb, :], ps)
                    nc.vector.tensor_scalar_add(xT[:, nb, :], xT[:, nb, :], projb[:, l, nb:nb + 1])

            # -- LN2 --
            h2_bf = actb.tile([128, S, TL], bf16, tag="h2")
            layer_norm(ln2w[:, l, :], ln2b[:, l, :], h2_bf)

            # -- MLP fc + gelu --
            g_bf = actb.tile([128, 4 * S, TL], bf16, tag="g")
            for c in range(4 * S // 2):
                wt = wf.tile([128, S, 256], bf16, tag="wf")
                nc.sync.dma_start(wt, w_fc[:, :, c * 256:(c + 1) * 256])
                for j in range(2):
                    nb = 2 * c + j
                    ps = ps_mm.tile([128, TL], f32, tag="mm")
                    for kt in range(S):
                        nc.tensor.matmul(ps, wt[:, kt, j * 128:j * 128 + 128], h2_bf[:, kt, :],
                                         start=(kt == 0), stop=(kt == S - 1))
                    nc.scalar.activation(g_bf[:, nb, :], ps, AF.Gelu_apprx_tanh,
                                         bias=fcb[:, l, nb:nb + 1])

            # -- MLP fco + residual --
            for c in range(3):
                wts = []
                for kc in range(2):
                    wt = wo.tile([128, 2 * S, 256], bf16, tag="wo")
                    nc.sync.dma_start(
                        wt, w_fco[:, kc * 2 * S:(kc + 1) * 2 * S, c * 256:(c + 1) * 256])
                    wts.append(wt)
                for j in range(2):
                    nb = 2 * c + j
                    ps = ps_mm.tile([128, TL], f32, tag="mm")
                    for kt in range(4 * S):
                        nc.tensor.matmul(ps, wts[kt // (2 * S)][:, kt % (2 * S), j * 128:j * 128 + 128],
                                         g_bf[:, kt, :],
                                         start=(kt == 0), stop=(kt == 4 * S - 1))
                    nc.vector.tensor_add(xT[:, nb, :], xT[:, nb, :], ps)
                    nc.vector.tensor_scalar_add(xT[:, nb, :], xT[:, nb, :], fcob[:, l, nb:nb + 1])

        # ---------------- final LN + gather + lm_head ----------------
        xf_bf = actb.tile([128, S, TL], bf16, tag="h")
        layer_norm(lnfw, lnfb, xf_bf)
        xf_in = dram_keep.tile([C, TL], bf16, tag="xf_in")
        nc.sync.dma_start(xf_in.rearrange("(s p) t -> p s t", **p128), xf_bf)
        xf_out = dram_keep.tile([NC * C, TL], bf16, tag="xf_out", addr_space="Shared")
        nc.gpsimd.collective_compute(
            "AllGather", OP.bypass, replica_groups=RG8,
            ins=[xf_in[:]], outs=[xf_out[:]])

        # close layer pools, open lm-head pools
        ctx.close()
        lm_sing = octx.enter_context(tc.tile_pool(name="lm_sing", bufs=1))
        lm_out = octx.enter_context(tc.tile_pool(name="lm_out", bufs=2))
        lm_ps = octx.enter_context(tc.tile_pool(name="lm_ps", bufs=6, space="PSUM"))

        wte_sb = lm_sing.tile([128, S, VSH], bf16)
        nc.sync.dma_start(wte_sb, tens["wteT"].rearrange("(s p) n -> p s n", **p128))
        xf_full = lm_sing.tile([128, NC * S, TL], bf16)
        nc.sync.dma_start(xf_full, xf_out.rearrange("(r s p) q -> p (r s) q", p=128, s=S))

        logits = tens["logits"]
        for tb in range(2 * NC):
            lo = lm_out.tile([128, VSH], f32, tag="lo")
            for vb in range(VSH // 512):
                ps = lm_ps.tile([128, 512], f32, tag="lm")
                for kt in range(S):
                    lhsT = xf_full[:, S * (tb // 2) + kt, (tb % 2) * 128:(tb % 2) * 128 + 128]
                    nc.tensor.matmul(ps, lhsT, wte_sb[:, kt, vb * 512:(vb + 1) * 512],
                                     start=(kt == 0), stop=(kt == S - 1))
                dst = lo[:, vb * 512:(vb + 1) * 512]
                if vb % 2 == 0:
                    nc.scalar.activation(dst, ps, AF.Copy)
                else:
                    nc.vector.tensor_copy(dst, ps)
            nc.sync.dma_start(logits[tb * 128:(tb + 1) * 128, :], lo)


def _build(n_layers):
    nc = bacc.Bacc("TRN2", target_bir_lowering=False, debug=False,
                   enable_asserts=False, num_devices=NC)
    tens = {}

    def din(name, shape, dtype):
        tens[name] = nc.dram_tensor(name, list(shape), dtype, kind="ExternalInput").ap()

    din("tokT", [C, TL], f32)
    din("wpeT", [C, TL], f32)
    din("mask", [128, KT, TL], bf16)
    din("wteT", [C, VSH], bf16)
    for l in range(n_layers):
        din(f"qkv_w_{l}", [C, 3 * C], bf16)
        din(f"proj_w_{l}", [C, C], bf16)
        din(f"fc_w_{l}", [C, 4 * C], bf16)
        din(f"fco_w_{l}", [4 * C, C], bf16)
    din("qkv_b", [n_layers, 3 * C], f32)
    din("proj_b", [n_layers, C], f32)
    din("fc_b", [n_layers, 4 * C], f32)
    din("fco_b", [n_layers, C], f32)
    din("ln1_w", [n_layers, C], f32)
    din("ln1_b", [n_layers, C], f32)
    din("ln2_w", [n_layers, C], f32)
    din("ln2_b", [n_layers, C], f32)
    din("lnf_w", [C], f32)
    din("lnf_b", [C], f32)
    tens["logits"] = nc.dram_tensor("logits", [B * T, VSH], f32, kind="ExternalOutput").ap()

    with tile.TileContext(nc) as tc:
        _emit(nc, tc, tens, n_layers)
    nc.compile()
    return nc


def _get_nc(n_layers):
    if n_layers not in _CACHE:
        _CACHE[n_layers] = _build(n_layers)
    return _CACHE[n_layers]


def _host_prep(inputs, n_layers):
    """Build per-core input maps from the full (unsharded) inputs."""
    bfl = ml_dtypes.bfloat16
    idx = np.asarray(inputs["idx"]).astype(np.int64)          # [B, T]
    wte = np.asarray(inputs["wte"], dtype=np.float32)          # [V, C]
    wpe = np.asarray(inputs["wpe"], dtype=np.float32)          # [T, C]

    tok = wte[idx.reshape(-1)]                                 # [B*T, C] f32 (gather only)
    pos = np.concatenate([wpe[:T]] * B, axis=0)                # [B*T, C]

    wteT = np.zeros((C, VSH * NC), dtype=bfl)
    wteT[:, :V] = wte.T.astype(bfl)

    shared = {}
    for l in range(n_layers):
        shared[f"qkv_w_{l}"] = np.ascontiguousarray(inputs["qkv_w"][l]).astype(bfl)
        shared[f"proj_w_{l}"] = np.ascontiguousarray(inputs["proj_w"][l]).astype(bfl)
        shared[f"fc_w_{l}"] = np.ascontiguousarray(inputs["fc_w"][l]).astype(bfl)
        shared[f"fco_w_{l}"] = np.ascontiguousarray(inputs["fco_w"][l]).astype(bfl)
    for name in ("qkv_b", "proj_b", "fc_b", "fco_b", "ln1_w", "ln1_b", "ln2_w", "ln2_b"):
        shared[name] = np.ascontiguousarray(inputs[name][:n_layers]).astype(np.float32)
    shared["lnf_w"] = np.asarray(inputs["lnf_w"], dtype=np.float32)
    shared["lnf_b"] = np.asarray(inputs["lnf_b"], dtype=np.float32)

    in_maps = []
    for c in range(NC):
        m = dict(shared)
        rows = slice(c * TL, (c + 1) * TL)
        m["tokT"] = np.ascontiguousarray(tok[rows].T)
        m["wpeT"] = np.ascontiguousarray(pos[rows].T)
        # mask[p, kt, q] = 1 if key (kt*128+p) <= query ((c%4)*256 + q)
        kg = np.arange(128)[:, None, None] + 128 * np.arange(KT)[None, :, None]
        qg = (c % 4) * TL + np.arange(TL)[None, None, :]
        m["mask"] = (kg <= qg).astype(bfl)
        m["wteT"] = np.ascontiguousarray(wteT[:, c * VSH:(c + 1) * VSH])
        in_maps.append(m)
    return in_maps


def _assemble(results):
    full = np.empty((B * T, NC * VSH), dtype=np.float32)
    for c in range(NC):
        full[:, c * VSH:(c + 1) * VSH] = results[c]["logits"]
    return np.ascontiguousarray(full[:, :V]).reshape(B, T, V)


def run(inputs, n_layers=L, trace=False, **kw):
    nc = _get_nc(n_layers)
    in_maps = _host_prep(inputs, n_layers)
    res = bass_utils.run_bass_kernel_spmd(
        nc, in_maps, core_ids=list(range(NC)), trace=trace, **kw)
    return _assemble(res.results), res


def kernel(**inputs) -> np.ndarray:
    out, _ = run(inputs)
    return out


# revision 17
# speedup vs baseline: 264.8030x; 264.8030x over previous
"""NanoGPT (GPT-2 124M) forward pass on 8 Trainium2 NeuronCores.

Sharding: token-parallel. B*T = 2048 tokens split into 8 contiguous chunks of
256; cores 0-3 hold batch 0, cores 4-7 hold batch 1. Weights are replicated
(bf16). Per layer each core computes qkv / attention / MLP for its 256 tokens;
K (feature-major) and V (token-major) are all-gathered within each 4-core
batch group so every core can attend over its full causal prefix (causal
structure applied via a per-core multiplicative mask input). lm_head is
vocab-parallel: after a final 8-rank all-gather of the lnf output, each core
computes logits for all 2048 tokens x a 6656-wide vocab shard of the (tied,
transposed) embedding matrix.

On-device layout: activations are feature-major [128 partitions, C/128, 256
tokens] so no transposes are ever needed; LayerNorm statistics are computed
with ones-vector matmuls (partition-axis sums), rsqrt via exp(-0.5*ln(v+eps)),
and per-token scalars are broadcast across partitions with gpsimd
partition_broadcast. Matmul inputs are bf16 (full PE rate); PSUM accumulation
and the residual stream are fp32.
"""

import os
import numpy as np
import ml_dtypes

import concourse.bass as bass
import concourse.mybir as mybir
import concourse.tile as tile
from concourse import bacc, bass_utils

# ---- model dims (hardcoded per problem spec) ----
B, T, C, H, L, V = 2, 1024, 768, 12, 12, 50257
D = C // H            # 64
NC = 8                # cores
TL = (B * T) // NC    # 256 local tokens per core
S = C // 128          # 6 feature sub-blocks
SQK = 2 * S           # 12 q+k feature blocks
KT = 1024 // 128      # 8 key-token blocks per batch
VSH = 6656            # padded vocab shard per core (13*512); 8*6656 >= V
EPS = 1e-5
ATT_SCALE = 1.0 / np.sqrt(D)

f32 = mybir.dt.float32
f32r = mybir.dt.float32r
bf16 = mybir.dt.bfloat16
AF = mybir.ActivationFunctionType
OP = mybir.AluOpType

RG4 = [[0, 1, 2, 3], [4, 5, 6, 7]]
RG8 = [[0, 1, 2, 3, 4, 5, 6, 7]]

_CACHE = {}


def _r(dt_np):
    return mybir.dt.from_np(np.dtype(dt_np))


def _emit(nc, tc, tens, n_layers):
    """Emit the full per-core program."""
    from contextlib import ExitStack

    with ExitStack() as octx, ExitStack() as ctx:
        dram_keep = octx.enter_context(tc.tile_pool(name="dram_keep", bufs=1, space="DRAM"))
        # ---------------- pools ----------------
        singles = ctx.enter_context(tc.tile_pool(name="singles", bufs=1))
        wq = ctx.enter_context(tc.tile_pool(name="wq", bufs=3))       # qk weight chunks
        wv = ctx.enter_context(tc.tile_pool(name="wv", bufs=2))       # v weight
        wp = ctx.enter_context(tc.tile_pool(name="wp", bufs=3))       # proj chunks
        wf = ctx.enter_context(tc.tile_pool(name="wf", bufs=3))       # fc chunks
        wo = ctx.enter_context(tc.tile_pool(name="wo", bufs=3))       # fco chunks
        act = ctx.enter_context(tc.tile_pool(name="act", bufs=1))     # f32 LN temps
        actb = ctx.enter_context(tc.tile_pool(name="actb", bufs=1))   # bf16 acts
        kv = ctx.enter_context(tc.tile_pool(name="kv", bufs=1))       # gathered K/V
        attp = ctx.enter_context(tc.tile_pool(name="attp", bufs=2))   # attT tiles
        small = ctx.enter_context(tc.tile_pool(name="small", bufs=8)) # [1,TL] stats
        bc = ctx.enter_context(tc.tile_pool(name="bc", bufs=2))       # bcast tiles
        ps_mm = ctx.enter_context(tc.tile_pool(name="ps_mm", bufs=3, space="PSUM"))
        ps_sc = ctx.enter_context(tc.tile_pool(name="ps_sc", bufs=2, space="PSUM"))
        ps_av = ctx.enter_context(tc.tile_pool(name="ps_av", bufs=1, space="PSUM"))
        dram = ctx.enter_context(tc.tile_pool(name="dram", bufs=2, space="DRAM"))

        # ---------------- resident constants ----------------
        def load_param(name, shape_dram_ap, sbuf_shape, dtype=f32):
            t = singles.tile(sbuf_shape, dtype, tag=name)
            nc.sync.dma_start(t, shape_dram_ap)
            return t

        p128 = dict(p=128)

        def load_param_l(name, dram, nblk):
            """[L, nblk*128] DRAM -> [128, L, nblk] SBUF, one DMA per layer."""
            t = singles.tile([128, n_layers, nblk], f32, tag=name)
            for l in range(n_layers):
                nc.sync.dma_start(t[:, l, :], dram[l].rearrange("(n p) -> p n", **p128))
            return t

        ln1w = load_param_l("ln1w", tens["ln1_w"], S)
        ln1b = load_param_l("ln1b", tens["ln1_b"], S)
        ln2w = load_param_l("ln2w", tens["ln2_w"], S)
        ln2b = load_param_l("ln2b", tens["ln2_b"], S)
        lnfw = load_param("lnfw", tens["lnf_w"].rearrange("(s p) -> p s", **p128), [128, S])
        lnfb = load_param("lnfb", tens["lnf_b"].rearrange("(s p) -> p s", **p128), [128, S])
        qkvb = load_param_l("qkvb", tens["qkv_b"][:, : 2 * C], SQK)
        vb_row = load_param("vb_row", tens["qkv_b"][None, :, 2 * C:], [1, n_layers, C])
        projb = load_param_l("projb", tens["proj_b"], S)
        fcb = load_param_l("fcb", tens["fc_b"], 4 * S)
        fcob = load_param_l("fcob", tens["fco_b"], S)

        mask_sb = singles.tile([128, KT, TL], bf16)
        nc.sync.dma_start(mask_sb, tens["mask"])

        ones_col = singles.tile([128, 1], bf16)
        nc.vector.memset(ones_col, 1.0)
        eps_t = singles.tile([1, 1], f32)
        nc.vector.memset(eps_t, EPS)

        # residual stream, feature-major fp32
        xT = singles.tile([128, S, TL], f32)
        # Vaug: [ktok 128, kt 8, head 12, 65] with ones column at [..,64]
        vaug = singles.tile([128, KT, H, D + 1], bf16)
        nc.vector.memset(vaug, 1.0)

        # ---------------- embedding ----------------
        tok_sb = act.tile([128, S, TL], f32, tag="tmp")
        nc.sync.dma_start(tok_sb, tens["tokT"].rearrange("(s p) t -> p s t", **p128))
        pos_sb = act.tile([128, S, TL], f32, tag="sq")
        nc.sync.dma_start(pos_sb, tens["wpeT"].rearrange("(s p) t -> p s t", **p128))
        nc.vector.tensor_add(xT, tok_sb, pos_sb)

        # ---------------- helpers ----------------
        def layer_norm(w_col, b_col, out_bf):
            """Feature-major LN of xT -> out_bf (bf16 [128,S,TL]).
            w_col/b_col: [128, S] per-partition scale/bias columns."""
            x_bf = act.tile([128, S, TL], bf16, tag="xbf")
            nc.vector.tensor_copy(x_bf, xT)
            sq = act.tile([128, S, TL], bf16, tag="sq")
            nc.vector.tensor_mul(sq, x_bf, x_bf)
            ps_s = ps_mm.tile([1, TL], f32, tag="mm")
            ps_q = ps_mm.tile([1, TL], f32, tag="mm")
            for kt in range(S):
                nc.tensor.matmul(ps_s, ones_col, x_bf[:, kt, :],
                                 start=(kt == 0), stop=(kt == S - 1))
            for kt in range(S):
                nc.tensor.matmul(ps_q, ones_col, sq[:, kt, :],
                                 start=(kt == 0), stop=(kt == S - 1))
            mean = small.tile([1, TL], f32, tag="st")
            nc.vector.tensor_scalar_mul(mean, ps_s, 1.0 / C)
            ex2 = small.tile([1, TL], f32, tag="st")
            nc.vector.tensor_scalar_mul(ex2, ps_q, 1.0 / C)
            var = small.tile([1, TL], f32, tag="st")
            nc.vector.tensor_mul(var, mean, mean)
            nc.vector.tensor_tensor(var, ex2, var, OP.subtract)
            lnv = small.tile([1, TL], f32, tag="st")
            nc.scalar.activation(lnv, var, AF.Ln, bias=eps_t)
            inv = small.tile([1, TL], f32, tag="st")
            nc.scalar.activation(inv, lnv, AF.Exp, scale=-0.5)
            mi = small.tile([1, TL], f32, tag="st")
            nc.vector.tensor_mul(mi, mean, inv)
            inv_b = bc.tile([128, TL], f32, tag="bc")
            nc.gpsimd.partition_broadcast(inv_b, inv)
            mi_b = bc.tile([128, TL], f32, tag="bc")
            nc.gpsimd.partition_broadcast(mi_b, mi)
            tmp = act.tile([128, S, TL], f32, tag="tmp")
            nc.vector.tensor_mul(tmp, xT, inv_b[:, None, :].to_broadcast((128, S, TL)))
            nc.vector.tensor_tensor(tmp, tmp, mi_b[:, None, :].to_broadcast((128, S, TL)), OP.subtract)
            for kt in range(S):
                nc.scalar.activation(out_bf[:, kt, :], tmp[:, kt, :], AF.Identity,
                                     bias=b_col[:, kt:kt + 1], scale=w_col[:, kt:kt + 1])

        # ---------------- transformer layers ----------------
        for l in range(n_layers):
            w_qkv = tens[f"qkv_w_{l}"].rearrange("(s p) n -> p s n", **p128)
            w_proj = tens[f"proj_w_{l}"].rearrange("(s p) n -> p s n", **p128)
            w_fc = tens[f"fc_w_{l}"].rearrange("(s p) n -> p s n", **p128)
            w_fco = tens[f"fco_w_{l}"].rearrange("(s p) n -> p s n", **p128)

            # -- LN1 --
            h_bf = actb.tile([128, S, TL], bf16, tag="h")
            layer_norm(ln1w[:, l, :], ln1b[:, l, :], h_bf)

            # -- qkv matmuls: K first (feeds the AllGather), then Q, then V --
            qkT = actb.tile([128, SQK, TL], bf16, tag="qk")
            for nb in list(range(S, SQK)) + list(range(S)):  # k blocks, then q
                wt = wq.tile([128, S, 256], bf16, tag="wq")
                c0 = (nb // 2) * 256
                nc.sync.dma_start(wt, w_qkv[:, :, c0:c0 + 256])
                ps = ps_mm.tile([128, TL], f32, tag="mm")
                off = (nb % 2) * 128
                for kt in range(S):
                    nc.tensor.matmul(ps, wt[:, kt, off:off + 128], h_bf[:, kt, :],
                                     start=(kt == 0), stop=(kt == S - 1))
                nc.scalar.activation(qkT[:, nb, :], ps, AF.Identity,
                                     bias=qkvb[:, l, nb:nb + 1])
                if nb == SQK - 1:
                    # kick off K all-gather as soon as k half is complete
                    k_in = dram.tile([C, TL], bf16, tag="k_in")
                    nc.sync.dma_start(k_in.rearrange("(s p) t -> p s t", **p128),
                                      qkT[:, S:SQK, :])
                    k_out = dram.tile([4 * C, TL], bf16, tag="k_out")
                    nc.gpsimd.collective_compute(
                        "AllGather", OP.bypass, replica_groups=RG4,
                        ins=[k_in[:]], outs=[k_out[:]])

            # V (token-major, form-1)
            v_sb = actb.tile([128, 2, C], bf16, tag="v")
            for vb in range(2):
                wtv = wv.tile([128, S, 384], bf16, tag="wv")
                nc.sync.dma_start(wtv, w_qkv[:, :, 2 * C + vb * 384:2 * C + (vb + 1) * 384])
                for tb in range(2):
                    ps = ps_mm.tile([128, 384], f32, tag="mm")
                    for kt in range(S):
                        nc.tensor.matmul(ps, h_bf[:, kt, tb * 128:(tb + 1) * 128],
                                         wtv[:, kt, :],
                                         start=(kt == 0), stop=(kt == S - 1))
                    nc.scalar.activation(v_sb[:, tb, vb * 384:(vb + 1) * 384], ps, AF.Copy)
            vb_b = bc.tile([128, C], f32, tag="vbias", bufs=1)
            nc.gpsimd.partition_broadcast(vb_b, vb_row[0:1, l, :])
            nc.vector.tensor_add(v_sb, v_sb, vb_b[:, None, :].to_broadcast((128, 2, C)))
            v_in = dram.tile([2 * 128, C], bf16, tag="v_in")
            nc.sync.dma_start(v_in.rearrange("(tb p) f -> p tb f", **p128), v_sb)
            v_out = dram.tile([8 * 128, C], bf16, tag="v_out")
            nc.gpsimd.collective_compute(
                "AllGather", OP.bypass, replica_groups=RG4,
                ins=[v_in[:]], outs=[v_out[:]])

            # -- load gathered K/V --
            k_full = kv.tile([128, 4 * S, TL], bf16, tag="kfull")
            nc.sync.dma_start(k_full, k_out.rearrange("(r s p) q -> p (r s) q", p=128, s=S))
            v_full = kv.tile([128, KT, C], bf16, tag="vfull")
            nc.sync.dma_start(v_full, v_out.rearrange("(kt p) f -> p kt f", **p128))
            for h in range(H):
                nc.vector.tensor_copy(vaug[:, :, h, 0:D], v_full[:, :, D * h:D * (h + 1)])

            # -- attention --
            y_bf = actb.tile([128, S, TL], bf16, tag="y")
            for h in range(H):
                po = 64 * (h % 2)
                sh = h // 2
                attT = attp.tile([128, KT, TL], bf16, tag="attT")
                for half in range(2):
                    ps = ps_sc.tile([128, 4, TL], f32, tag="sc")
                    for j in range(4):
                        kt = half * 4 + j
                        lhsT = k_full[po:po + 64, S * (kt // 2) + sh, (kt % 2) * 128:(kt % 2) * 128 + 128]
                        rhs = qkT[po:po + 64, sh, :]
                        nc.tensor.matmul(ps[:, j, :], lhsT, rhs, start=True, stop=True)
                    nc.scalar.activation(attT[:, half * 4:half * 4 + 4, :], ps, AF.Exp,
                                         scale=ATT_SCALE)
                nc.vector.tensor_mul(attT, attT, mask_sb)
                ps_y = ps_av.tile([D + 1, TL], f32, tag="av")
                for kt in range(KT):
                    nc.tensor.matmul(ps_y, vaug[:, kt, h, :], attT[:, kt, :],
                                     start=(kt == 0), stop=(kt == KT - 1))
                inv_h = small.tile([1, TL], f32, tag="st")
                nc.vector.reciprocal(inv_h, ps_y[D:D + 1, :])
                inv_hb = bc.tile([64, TL], f32, tag="bch")
                nc.gpsimd.partition_broadcast(inv_hb, inv_h)
                nc.vector.tensor_mul(y_bf[po:po + 64, sh, :], ps_y[0:D, :], inv_hb)

            # -- proj + residual --
            for c in range(3):
                wt = wp.tile([128, S, 256], bf16, tag="wp")
                nc.sync.dma_start(wt, w_proj[:, :, c * 256:(c + 1) * 256])
                for j in range(2):
                    nb = 2 * c + j
                    ps = ps_mm.tile([128, TL], f32, tag="mm")
                    for kt in range(S):
                        nc.tensor.matmul(ps, wt[:, kt, j * 128:j * 128 + 128], y_bf[:, kt, :],
                                         start=(kt == 0), stop=(kt == S - 1))
                    nc.vector.tensor_add(xT[:, nb, :], xT[:, nb, :], ps)
                    nc.vector.tensor_scalar_add(xT[:, nb, :], xT[:, nb, :], projb[:, l, nb:nb + 1])

            # -- LN2 --
            h2_bf = actb.tile([128, S, TL], bf16, tag="h2")
            layer_norm(ln2w[:, l, :], ln2b[:, l, :], h2_bf)

            # -- MLP fc + gelu --
            g_bf = actb.tile([128, 4 * S, TL], bf16, tag="g")
            for c in range(4 * S // 2):
                wt = wf.tile([128, S, 256], bf16, tag="wf")
                nc.sync.dma_start(wt, w_fc[:, :, c * 256:(c + 1) * 256])
                for j in range(2):
                    nb = 2 * c + j
                    ps = ps_mm.tile([128, TL], f32, tag="mm")
                    for kt in range(S):
                        nc.tensor.matmul(ps, wt[:, kt, j * 128:j * 128 + 128], h2_bf[:, kt, :],
                                         start=(kt == 0), stop=(kt == S - 1))
                    nc.scalar.activation(g_bf[:, nb, :], ps, AF.Gelu_apprx_tanh,
                                         bias=fcb[:, l, nb:nb + 1])

            # -- MLP fco + residual --
            for c in range(3):
                wts = []
                for kc in range(2):
                    wt = wo.tile([128, 2 * S, 256], bf16, tag="wo")
                    nc.sync.dma_start(
                        wt, w_fco[:, kc * 2 * S:(kc + 1) * 2 * S, c * 256:(c + 1) * 256])
                    wts.append(wt)
                for j in range(2):
                    nb = 2 * c + j
                    ps = ps_mm.tile([128, TL], f32, tag="mm")
                    for kt in range(4 * S):
                        nc.tensor.matmul(ps, wts[kt // (2 * S)][:, kt % (2 * S), j * 128:j * 128 + 128],
                                         g_bf[:, kt, :],
                                         start=(kt == 0), stop=(kt == 4 * S - 1))
                    nc.vector.tensor_add(xT[:, nb, :], xT[:, nb, :], ps)
                    nc.vector.tensor_scalar_add(xT[:, nb, :], xT[:, nb, :], fcob[:, l, nb:nb + 1])

        # ---------------- final LN + gather + lm_head ----------------
        xf_bf = actb.tile([128, S, TL], bf16, tag="h")
        layer_norm(lnfw, lnfb, xf_bf)
        xf_in = dram_keep.tile([C, TL], bf16, tag="xf_in")
        nc.sync.dma_start(xf_in.rearrange("(s p) t -> p s t", **p128), xf_bf)
        xf_out = dram_keep.tile([NC * C, TL], bf16, tag="xf_out", addr_space="Shared")
        nc.gpsimd.collective_compute(
            "AllGather", OP.bypass, replica_groups=RG8,
            ins=[xf_in[:]], outs=[xf_out[:]])

        # close layer pools, open lm-head pools
        ctx.close()
        lm_sing = octx.enter_context(tc.tile_pool(name="lm_sing", bufs=1))
        lm_out = octx.enter_context(tc.tile_pool(name="lm_out", bufs=2))
        lm_ps = octx.enter_context(tc.tile_pool(name="lm_ps", bufs=6, space="PSUM"))

        wte_sb = lm_sing.tile([128, S, VSH], bf16)
        nc.sync.dma_start(wte_sb, tens["wteT"].rearrange("(s p) n -> p s n", **p128))
        xf_full = lm_sing.tile([128, NC * S, TL], bf16)
        nc.sync.dma_start(xf_full, xf_out.rearrange("(r s p) q -> p (r s) q", p=128, s=S))

        logits = tens["logits"]
        for tb in range(2 * NC):
            lo = lm_out.tile([128, VSH], f32, tag="lo")
            for vb in range(VSH // 512):
                ps = lm_ps.tile([128, 512], f32, tag="lm")
                for kt in range(S):
                    lhsT = xf_full[:, S * (tb // 2) + kt, (tb % 2) * 128:(tb % 2) * 128 + 128]
                    nc.tensor.matmul(ps, lhsT, wte_sb[:, kt, vb * 512:(vb + 1) * 512],
                                     start=(kt == 0), stop=(kt == S - 1))
                dst = lo[:, vb * 512:(vb + 1) * 512]
                if vb % 2 == 0:
                    nc.scalar.activation(dst, ps, AF.Copy)
                else:
                    nc.vector.tensor_copy(dst, ps)
            nc.sync.dma_start(logits[tb * 128:(tb + 1) * 128, :], lo)


def _build(n_layers):
    nc = bacc.Bacc("TRN2", target_bir_lowering=False, debug=False,
                   enable_asserts=False, num_devices=NC)
    tens = {}

    def din(name, shape, dtype):
        tens[name] = nc.dram_tensor(name, list(shape), dtype, kind="ExternalInput").ap()

    din("tokT", [C, TL], f32)
    din("wpeT", [C, TL], f32)
    din("mask", [128, KT, TL], bf16)
    din("wteT", [C, VSH], bf16)
    for l in range(n_layers):
        din(f"qkv_w_{l}", [C, 3 * C], bf16)
        din(f"proj_w_{l}", [C, C], bf16)
        din(f"fc_w_{l}", [C, 4 * C], bf16)
        din(f"fco_w_{l}", [4 * C, C], bf16)
    din("qkv_b", [n_layers, 3 * C], f32)
    din("proj_b", [n_layers, C], f32)
    din("fc_b", [n_layers, 4 * C], f32)
    din("fco_b", [n_layers, C], f32)
    din("ln1_w", [n_layers, C], f32)
    din("ln1_b", [n_layers, C], f32)
    din("ln2_w", [n_layers, C], f32)
    din("ln2_b", [n_layers, C], f32)
    din("lnf_w", [C], f32)
    din("lnf_b", [C], f32)
    tens["logits"] = nc.dram_tensor("logits", [B * T, VSH], f32, kind="ExternalOutput").ap()

    with tile.TileContext(nc) as tc:
        _emit(nc, tc, tens, n_layers)
    nc.compile()
    return nc


def _get_nc(n_layers):
    if n_layers not in _CACHE:
        _CACHE[n_layers] = _build(n_layers)
    return _CACHE[n_layers]


def _host_prep(inputs, n_layers):
    """Build per-core input maps from the full (unsharded) inputs."""
    bfl = ml_dtypes.bfloat16
    idx = np.asarray(inputs["idx"]).astype(np.int64)          # [B, T]
    wte = np.asarray(inputs["wte"], dtype=np.float32)          # [V, C]
    wpe = np.asarray(inputs["wpe"], dtype=np.float32)          # [T, C]

    tok = wte[idx.reshape(-1)]                                 # [B*T, C] f32 (gather only)
    pos = np.concatenate([wpe[:T]] * B, axis=0)                # [B*T, C]

    wteT = np.zeros((C, VSH * NC), dtype=bfl)
    wteT[:, :V] = wte.T.astype(bfl)

    shared = {}
    for l in range(n_layers):
        shared[f"qkv_w_{l}"] = np.ascontiguousarray(inputs["qkv_w"][l]).astype(bfl)
        shared[f"proj_w_{l}"] = np.ascontiguousarray(inputs["proj_w"][l]).astype(bfl)
        shared[f"fc_w_{l}"] = np.ascontiguousarray(inputs["fc_w"][l]).astype(bfl)
        shared[f"fco_w_{l}"] = np.ascontiguousarray(inputs["fco_w"][l]).astype(bfl)
    for name in ("qkv_b", "proj_b", "fc_b", "fco_b", "ln1_w", "ln1_b", "ln2_w", "ln2_b"):
        shared[name] = np.ascontiguousarray(inputs[name][:n_layers]).astype(np.float32)
    shared["lnf_w"] = np.asarray(inputs["lnf_w"], dtype=np.float32)
    shared["lnf_b"] = np.asarray(inputs["lnf_b"], dtype=np.float32)

    in_maps = []
    for c in range(NC):
        m = dict(shared)
        rows = slice(c * TL, (c + 1) * TL)
        m["tokT"] = np.ascontiguousarray(tok[rows].T)
        m["wpeT"] = np.ascontiguousarray(pos[rows].T)
        # mask[p, kt, q] = 1 if key (kt*128+p) <= query ((c%4)*256 + q)
        kg = np.arange(128)[:, None, None] + 128 * np.arange(KT)[None, :, None]
        qg = (c % 4) * TL + np.arange(TL)[None, None, :]
        m["mask"] = (kg <= qg).astype(bfl)
        m["wteT"] = np.ascontiguousarray(wteT[:, c * VSH:(c + 1) * VSH])
        in_maps.append(m)
    return in_maps


def _assemble(results):
    full = np.empty((B * T, NC * VSH), dtype=np.float32)
    for c in range(NC):
        full[:, c * VSH:(c + 1) * VSH] = results[c]["logits"]
    return np.ascontiguousarray(full[:, :V]).reshape(B, T, V)


def run(inputs, n_layers=L, trace=False, **kw):
    nc = _get_nc(n_layers)
    in_maps = _host_prep(inputs, n_layers)
    res = bass_utils.run_bass_kernel_spmd(
        nc, in_maps, core_ids=list(range(NC)), trace=trace, **kw)
    return _assemble(res.results), res


def kernel(**inputs) -> np.ndarray:
    out, _ = run(inputs)
    return out
